# revision 16
# baseline (speedup 1.0000x reference)
"""GQA attention forward on 8 TRN2 NeuronCores, tensor-parallel across heads.

Problem (hardcoded): B=2, T=2048, D=2048, 16 q-heads, 4 kv-heads, head_dim=128,
RoPE (rotate-half pairing i <-> i+64), causal softmax, output projection.

Sharding (per core c of 8):
  q-heads 2c, 2c+1 (rows 256c:256c+256 of wq), kv-head c//2 (rows of wk/wv),
  wo input-dim slice [:, 256c:256c+256]. x replicated. Each core computes a
  full-shape partial of the output (y_local @ wo_slice.T); host sums partials.

v3 design notes:
  - fp16 activations/weights (f32 PSUM): all tensors here are O(100) so fp16's
    4x-finer mantissa beats bf16 at identical PE/DMA cost, and 16-bit DVE ops
    run in 2x mode. CPU-sim rel err 7e-4 (max exp value ~1.1e3 << 65504).
  - Host packs every DRAM tensor partition-major so each DMA is 128 contiguous
    descriptors (the naive [D, features] layouts produced 256B descriptors
    that made weight loads 5-9us each).
  - Fused pipeline per 512-token block: proj -> attention, with the previous
    block's out-projection emitted one jb-chunk at a time BETWEEN attention
    j-tiles (attention alone is exp-throughput-bound on ACT at ~690ns/tile vs
    the PE's ~430ns/tile, so out-proj matmuls fill the PE bubbles).
  - Softmax denominator: est tiles accumulate elementwise into two fp16
    chains (even/odd j-tiles, halving the serial DVE latency), then ONE
    all-ones matmul per chain fuses the partition-reduce AND the broadcast
    (every output row = column sums) into a 512-cycle PE op. reciprocal via
    the approx-fast custom DVE op. No gpsimd in the chain (its library swaps
    between op types cost ~15us stalls in v2); gpsimd only runs the rope cos
    muls (single op type, single library).
  - PSUM = exactly 8 banks: pj(2: q0/q1/k/v/vtr ring), pst(2: rope-swap +
    score tiles + denom ring), py(2), po(2, jb ping-pong in halves).
"""
import math
import numpy as np

P = 128
B = 2
T = 2048
D = 2048
BT = B * T            # 4096
HD = 128              # head dim
QH = 2                # local q heads per core
KT = D // P           # 16 contraction tiles over D
NB = 512              # free-dim block (tokens)
IB = T // NB          # 4 i-blocks per batch
NJT_MAX = T // P      # 16 j-tiles per batch
NCORES = 8
SCALE = 1.0 / math.sqrt(HD)

_CACHE = {}


def _build():
    import concourse.bass as bass
    import concourse.mybir as mybir
    from concourse import bacc
    from concourse.tile import TileContext

    F32 = mybir.dt.float32
    F16 = mybir.dt.float16
    EXP = mybir.ActivationFunctionType.Exp

    nc = bacc.Bacc("TRN2", target_bir_lowering=False, debug=False)

    # all inputs partition-major-packed on host: [128, ...] contiguous rows
    x_d = nc.dram_tensor("xp", [P, 2 * IB * KT * NB], F16, kind="ExternalInput").ap()
    wqkv_d = nc.dram_tensor("wqkv", [P, KT * 4 * HD], F16, kind="ExternalInput").ap()
    wo_d = nc.dram_tensor("wop", [P, QH * D], F16, kind="ExternalInput").ap()
    cb_d = nc.dram_tensor("cb", [P, 4 * P], F16, kind="ExternalInput").ap()
    cf_d = nc.dram_tensor("cf", [P, 2 * T], F32, kind="ExternalInput").ap()
    out_d = nc.dram_tensor("out", [BT, D], F16, kind="ExternalOutput").ap()

    NHB = 2 * IB * 2   # 16 half-blocks of 8 kt-tiles each
    x_r = x_d.rearrange("p (hb kt m) -> p hb kt m", hb=NHB, kt=KT // 2)
    wqkv_r = wqkv_d.rearrange("p (h kt m) -> p h kt m", h=2, kt=KT // 2)
    wo_r = wo_d.rearrange("p (h j) -> p h j", h=QH)
    cb_r = cb_d.rearrange("p (a q) -> p a q", a=4)
    cf_r = cf_d.rearrange("p (a t) -> p a t", a=2)

    with TileContext(nc) as tc:
        with (
            tc.tile_pool(name="consts", bufs=1) as consts,
            tc.tile_pool(name="acts", bufs=1) as acts,
            tc.tile_pool(name="xt", bufs=4) as xt_pool,
            tc.tile_pool(name="qr", bufs=2) as qr_pool,
            tc.tile_pool(name="raw", bufs=3) as raw_pool,
            tc.tile_pool(name="tt", bufs=2) as t_pool,
            tc.tile_pool(name="est", bufs=4) as est_pool,
            tc.tile_pool(name="accp", bufs=4) as acc_pool,
            tc.tile_pool(name="rinv", bufs=2) as rinv_pool,
            tc.tile_pool(name="ysb", bufs=2) as y_pool,
            tc.tile_pool(name="osb", bufs=2) as o_pool,
            tc.tile_pool(name="pj", bufs=2, space="PSUM") as pj,
            tc.tile_pool(name="pst", bufs=1, space="PSUM") as pst,
            tc.tile_pool(name="py", bufs=2, space="PSUM") as py,
            tc.tile_pool(name="po", bufs=1, space="PSUM") as po,
        ):
            # ---- resident constants / weights (ACT HWDGE queue) ----
            cb_sb = consts.tile([P, 4, P], F16)
            wqkv_sb = consts.tile([P, 2, KT // 2, 4 * HD], F16)
            cs_sb = consts.tile([P, 2, T], F32)
            wo_sb = consts.tile([P, QH, D], F16)
            nc.scalar.dma_start(wqkv_sb[:, 0], wqkv_r[:, 0])
            nc.scalar.dma_start(wqkv_sb[:, 1], wqkv_r[:, 1])
            nc.scalar.dma_start(cs_sb, cf_r)
            nc.scalar.dma_start(wo_sb, wo_r)
            perm = cb_sb[:, 0, :]
            triu = cb_sb[:, 1, :]
            ident = cb_sb[:, 2, :]
            aones = cb_sb[:, 3, :]
            cos_t = cs_sb[:, 0, :]
            sin_t = cs_sb[:, 1, :]

            def wslc(kt, c0, c1):
                return wqkv_sb[:, kt // 8, kt % 8, c0:c1]

            # ---- resident activations (per-batch slots) ----
            kr_sb = acts.tile([P, B, T], F16)
            vt_sb = acts.tile([P, B, NJT_MAX, HD], F16)

            xt_tiles = {}

            def prefetch(hb):
                if hb >= NHB or hb in xt_tiles:
                    return
                xt = xt_pool.tile([P, KT // 2, NB], F16, tag="xt", name="xt")
                nc.sync.dma_start(xt, x_r[:, hb])
                xt_tiles[hb] = xt

            def rope(ps_raw, dst, t0):
                # dst(fp16) = raw*cos + swap(raw)*ssin; swap via PE perm matmul
                raw = raw_pool.tile([P, NB], F16, tag="raw")
                nc.scalar.copy(raw, ps_raw)  # frees the psum bank quickly
                t1 = t_pool.tile([P, NB], F32, tag="t1")
                nc.gpsimd.tensor_mul(t1, raw, cos_t[:, t0:t0 + NB])
                ps_sw = pj.tile([P, NB], F32, tag="pj", name="ps_sw")
                nc.tensor.matmul(ps_sw, perm, raw, start=True, stop=True)
                t2 = t_pool.tile([P, NB], F32, tag="t2")
                nc.vector.tensor_mul(t2, ps_sw, sin_t[:, t0:t0 + NB])
                nc.vector.tensor_add(dst, t1, t2)

            def make_outproj_steps(i0p, y_prev, po_tiles):
                steps = []
                state = {}

                def step(s, jb):
                    def run(in_attn=True):
                        bank = po_tiles[(s * (D // NB) + jb) % len(po_tiles)]
                        if jb == 0:
                            state["o"] = o_pool.tile([P, D], F16, tag="o",
                                                     name="o_sb")
                        o_sb = state["o"]
                        nc.tensor.matmul(
                            bank,
                            y_prev[:, 0, s * P:(s + 1) * P],
                            wo_sb[:, 0, jb * NB:(jb + 1) * NB],
                            start=True, stop=False,
                        )
                        nc.tensor.matmul(
                            bank,
                            y_prev[:, 1, s * P:(s + 1) * P],
                            wo_sb[:, 1, jb * NB:(jb + 1) * NB],
                            start=False, stop=True,
                        )
                        dst = o_sb[:, jb * NB:(jb + 1) * NB]
                        # interleaved in attention ACT is exp-bound: give it
                        # only 1-in-4 copies; in bare tail flushes alternate
                        # DVE/ACT so the po ring drains at matmul rate
                        on_act = jb == 1 if in_attn else jb % 2 == 1
                        if on_act:
                            nc.scalar.copy(dst, bank)
                        else:
                            nc.vector.tensor_copy(dst, bank)
                        if jb == D // NB - 1:
                            row0 = i0p + s * P
                            nc.sync.dma_start(out_d[row0:row0 + P, :], o_sb)
                    return run

                for s in range(NB // P):
                    for jb in range(D // NB):
                        steps.append(step(s, jb))
                return steps

            def emit_proj(b, ib, gblk):
                xta = xt_tiles.pop(2 * gblk)
                xtb = xt_tiles.pop(2 * gblk + 1)
                prefetch(2 * gblk + 4)
                prefetch(2 * gblk + 5)
                t0 = ib * NB

                def xthalf(kt):
                    return (xta if kt < 8 else xtb)[:, kt % 8, :]

                # pass A: the two local q heads
                ps_q0 = pj.tile([P, NB], F32, tag="pj", name="ps_q0")
                ps_q1 = pj.tile([P, NB], F32, tag="pj", name="ps_q1")
                for kt in range(KT):
                    st, sp = kt == 0, kt == KT - 1
                    nc.tensor.matmul(ps_q0, wslc(kt, 0, P), xthalf(kt),
                                     start=st, stop=sp)
                    nc.tensor.matmul(ps_q1, wslc(kt, P, 2 * P), xthalf(kt),
                                     start=st, stop=sp)
                qr = qr_pool.tile([P, QH, NB], F16, tag="qr", name="qr")
                rope(ps_q0, qr[:, 0, :], t0)
                rope(ps_q1, qr[:, 1, :], t0)
                # pass B: k and v for the local kv head
                ps_k = pj.tile([P, NB], F32, tag="pj", name="ps_k")
                ps_v = pj.tile([P, NB], F32, tag="pj", name="ps_v")
                for kt in range(KT):
                    st, sp = kt == 0, kt == KT - 1
                    nc.tensor.matmul(ps_k, wslc(kt, 2 * P, 3 * P), xthalf(kt),
                                     start=st, stop=sp)
                    nc.tensor.matmul(ps_v, wslc(kt, 3 * P, 4 * P), xthalf(kt),
                                     start=st, stop=sp)
                rope(ps_k, kr_sb[:, b, ib * NB:(ib + 1) * NB], t0)
                vraw = raw_pool.tile([P, NB], F16, tag="raw", name="vraw")
                nc.scalar.copy(vraw, ps_v)
                ps_tr = pj.tile([P, 4, P], F16, tag="pj", name="ps_tr")
                for s4 in range(4):
                    nc.tensor.transpose(ps_tr[:, s4, :],
                                        vraw[:, s4 * P:(s4 + 1) * P], ident)
                nc.vector.tensor_copy(vt_sb[:, b, ib * 4:(ib + 1) * 4, :], ps_tr)
                return qr

            def emit_attn(b, ib, qr, steps):
                y_sb = y_pool.tile([P, QH, NB], F16, tag="y", name="y_sb")
                njt = 4 * ib + 4
                for h in range(QH):
                    ps_y = py.tile([P, NB], F32, tag="py", name="ps_y")
                    acc0 = acc_pool.tile([P, NB], F16, tag="acc", name="acc0")
                    acc1 = acc_pool.tile([P, NB], F16, tag="acc", name="acc1")
                    for g in range(njt // 2):
                        ps2 = pst.tile([P, 2, NB], F32, tag="st", name="ps_st")
                        est2 = est_pool.tile([P, 2, NB], F16, tag="est",
                                             name="est")
                        subs = []
                        for u in range(2):
                            jt = 2 * g + u
                            sub = max(0, jt - 4 * ib) * P
                            subs.append(sub)
                            nc.tensor.matmul(
                                ps2[:, u, sub:],
                                kr_sb[:, b, jt * P:(jt + 1) * P],
                                qr[:, h, sub:],
                                start=True, stop=True,
                            )
                        # one exp for the pair; garbage in masked-off regions
                        # of diagonal tiles is never read downstream
                        nc.scalar.activation(est2, ps2, EXP, scale=SCALE)
                        for u in range(2):
                            jt = 2 * g + u
                            sub = subs[u]
                            est = est2[:, u, :]
                            if jt - 4 * ib >= 0:  # diagonal: triangle mask
                                nc.vector.tensor_mul(est[:, sub:sub + P],
                                                     est[:, sub:sub + P], triu)
                            acc = acc0 if u == 0 else acc1
                            if jt < 2:  # first tile of this chain
                                if sub > 0:
                                    nc.vector.memset(acc[:, 0:sub], 0.0)
                                nc.vector.tensor_copy(acc[:, sub:], est[:, sub:])
                            else:
                                nc.vector.tensor_add(acc[:, sub:], acc[:, sub:],
                                                     est[:, sub:])
                            nc.tensor.matmul(
                                ps_y[:, sub:],
                                vt_sb[:, b, jt, :],
                                est[:, sub:],
                                start=jt == 0, stop=jt == njt - 1,
                            )
                            if steps:
                                steps.pop(0)()
                    # fused partition-reduce + broadcast: every row of the
                    # all-ones matmul output is the per-column denominator
                    rb_ps = py.tile([P, NB], F32, tag="py", name="rb_ps")
                    nc.tensor.matmul(rb_ps, aones, acc0, start=True, stop=False)
                    nc.tensor.matmul(rb_ps, aones, acc1, start=False, stop=True)
                    rinv = rinv_pool.tile([P, NB], F32, tag="rinv", name="rinv")
                    nc.vector.reciprocal_approx_fast(rinv, rb_ps)
                    nc.vector.tensor_mul(y_sb[:, h, :], ps_y, rinv)
                return y_sb

            prefetch(0)
            prefetch(1)
            nc.sync.dma_start(cb_sb, cb_r)
            prefetch(2)
            prefetch(3)
            steps = []
            for b in range(B):
                for ib in range(IB):
                    gblk = b * IB + ib
                    qr = emit_proj(b, ib, gblk)
                    y_sb = emit_attn(b, ib, qr, steps)
                    for f in steps:  # leftovers (small-ib blocks)
                        f(in_attn=False)
                    po_t = po.tile([P, 2, NB], F32, tag="po", name="po_t")
                    po_tiles = [po_t[:, 0, :], po_t[:, 1, :]]
                    if (b, ib) == (B - 1, IB - 1):
                        # last block: projections are done, borrow the pj ring
                        # so the bare final out-proj rotates over 4 banks
                        po_tiles.append(pj.tile([P, NB], F32, tag="pj",
                                                name="po_t2"))
                        po_tiles.append(pj.tile([P, NB], F32, tag="pj",
                                                name="po_t3"))
                    steps = make_outproj_steps(b * T + ib * NB, y_sb, po_tiles)
            for f in steps:
                f(in_attn=False)

    nc.compile()
    return nc


def _host_prep(x, rope, wq, wk, wv, wo):
    """Build the 8 per-core input maps: shard, fp16, partition-major pack."""
    f16 = np.float16
    xT = x.reshape(BT, D).T.astype(f16)                 # [D, BT]
    xp = np.ascontiguousarray(
        xT.reshape(KT, P, 2 * IB, NB).transpose(1, 2, 0, 3).reshape(P, -1))
    cos = np.asarray(rope[..., 0], dtype=np.float32)    # [T, 64]
    sin = np.asarray(rope[..., 1], dtype=np.float32)
    cosT = np.concatenate([cos.T, cos.T], axis=0)       # [128, T]
    ssinT = np.concatenate([-sin.T, sin.T], axis=0)
    cf = np.ascontiguousarray(np.concatenate([cosT, ssinT], axis=1))
    permm = np.zeros((P, P), dtype=np.float32)
    permm[(np.arange(P) + 64) % P, np.arange(P)] = 1.0
    triu = np.triu(np.ones((P, P), dtype=np.float32))
    ident = np.eye(P, dtype=np.float32)
    aones = np.ones((P, P), dtype=np.float32)
    cb = np.ascontiguousarray(
        np.concatenate([permm, triu, ident, aones], axis=1).astype(f16))

    in_maps = []
    for c in range(NCORES):
        kv = c // 2
        wqkv = np.concatenate(
            [wq[QH * HD * c:QH * HD * (c + 1), :].T,
             wk[HD * kv:HD * (kv + 1), :].T,
             wv[HD * kv:HD * (kv + 1), :].T], axis=1).astype(f16)  # [D, 512]
        wqkv_p = np.ascontiguousarray(
            wqkv.reshape(KT, P, 4 * HD).transpose(1, 0, 2).reshape(P, -1))
        woT = wo[:, QH * HD * c:QH * HD * (c + 1)].T.astype(f16)   # [256, D]
        wo_p = np.ascontiguousarray(
            woT.reshape(QH, P, D).transpose(1, 0, 2).reshape(P, -1))
        in_maps.append(
            {"xp": xp, "wqkv": wqkv_p, "wop": wo_p, "cb": cb, "cf": cf}
        )
    return in_maps


LAST_RESULTS = None


def kernel(x, rope, wq, wk, wv, wo):
    global LAST_RESULTS
    from concourse import bass_utils

    if "nc" not in _CACHE:
        _CACHE["nc"] = _build()
    nc = _CACHE["nc"]

    in_maps = _host_prep(
        np.asarray(x), np.asarray(rope), np.asarray(wq), np.asarray(wk),
        np.asarray(wv), np.asarray(wo)
    )
    res = bass_utils.run_bass_kernel_spmd(nc, in_maps, core_ids=list(range(NCORES)))
    LAST_RESULTS = res
    acc = np.zeros((BT, D), dtype=np.float64)
    for c in range(NCORES):
        acc += res.results[c]["out"].astype(np.float64)
    return acc.reshape(B, T, D).astype(np.float32)


# revision 18
# speedup vs baseline: 1.0717x; 1.0717x over previous
"""GQA attention forward on 8 TRN2 NeuronCores, tensor-parallel across heads.

Problem (hardcoded): B=2, T=2048, D=2048, 16 q-heads, 4 kv-heads, head_dim=128,
RoPE (rotate-half pairing i <-> i+64), causal softmax, output projection.

Sharding (per core c of 8):
  q-heads 2c, 2c+1 (rows 256c:256c+256 of wq), kv-head c//2 (rows of wk/wv),
  wo input-dim slice [:, 256c:256c+256]. x replicated. Each core computes a
  full-shape partial of the output (y_local @ wo_slice.T); host sums partials.

v3 design notes:
  - fp16 activations/weights (f32 PSUM): all tensors here are O(100) so fp16's
    4x-finer mantissa beats bf16 at identical PE/DMA cost, and 16-bit DVE ops
    run in 2x mode. CPU-sim rel err 7e-4 (max exp value ~1.1e3 << 65504).
  - Host packs every DRAM tensor partition-major so each DMA is 128 contiguous
    descriptors (the naive [D, features] layouts produced 256B descriptors
    that made weight loads 5-9us each).
  - Fused pipeline per 512-token block: proj -> attention, with the previous
    block's out-projection emitted one jb-chunk at a time BETWEEN attention
    j-tiles (attention alone is exp-throughput-bound on ACT at ~690ns/tile vs
    the PE's ~430ns/tile, so out-proj matmuls fill the PE bubbles).
  - Softmax denominator: est tiles accumulate elementwise into two fp16
    chains (even/odd j-tiles, halving the serial DVE latency), then ONE
    all-ones matmul per chain fuses the partition-reduce AND the broadcast
    (every output row = column sums) into a 512-cycle PE op. reciprocal via
    the approx-fast custom DVE op. No gpsimd in the chain (its library swaps
    between op types cost ~15us stalls in v2); gpsimd only runs the rope cos
    muls (single op type, single library).
  - PSUM = exactly 8 banks: pj(2: q0/q1/k/v/vtr ring), pst(2: rope-swap +
    score tiles + denom ring), py(2), po(2, jb ping-pong in halves).
"""
import math
import numpy as np

P = 128
B = 2
T = 2048
D = 2048
BT = B * T            # 4096
HD = 128              # head dim
QH = 2                # local q heads per core
KT = D // P           # 16 contraction tiles over D
NB = 512              # free-dim block (tokens)
IB = T // NB          # 4 i-blocks per batch
NJT_MAX = T // P      # 16 j-tiles per batch
NCORES = 8
SCALE = 1.0 / math.sqrt(HD)

_CACHE = {}


def _build():
    import concourse.bass as bass
    import concourse.mybir as mybir
    from concourse import bacc
    from concourse.tile import TileContext

    F32 = mybir.dt.float32
    F16 = mybir.dt.float16
    EXP = mybir.ActivationFunctionType.Exp

    nc = bacc.Bacc("TRN2", target_bir_lowering=False, debug=False)

    # all inputs partition-major-packed on host: [128, ...] contiguous rows
    x_d = nc.dram_tensor("xp", [P, 2 * IB * KT * NB], F16, kind="ExternalInput").ap()
    wqkv_d = nc.dram_tensor("wqkv", [P, KT * 4 * HD], F16, kind="ExternalInput").ap()
    wo_d = nc.dram_tensor("wop", [P, QH * D], F16, kind="ExternalInput").ap()
    cb_d = nc.dram_tensor("cb", [P, 4 * P], F16, kind="ExternalInput").ap()
    cf_d = nc.dram_tensor("cf", [P, 2 * T], F32, kind="ExternalInput").ap()
    out_d = nc.dram_tensor("out", [BT, D], F16, kind="ExternalOutput").ap()

    NHB = 2 * IB * 2   # 16 half-blocks of 8 kt-tiles each
    x_r = x_d.rearrange("p (hb kt m) -> p hb kt m", hb=NHB, kt=KT // 2)
    wqkv_r = wqkv_d.rearrange("p (h kt m) -> p h kt m", h=2, kt=KT // 2)
    wo_r = wo_d.rearrange("p (h j) -> p h j", h=QH)
    cb_r = cb_d.rearrange("p (a q) -> p a q", a=4)
    cf_r = cf_d.rearrange("p (a t) -> p a t", a=2)

    with TileContext(nc) as tc:
        with (
            tc.tile_pool(name="consts", bufs=1) as consts,
            tc.tile_pool(name="acts", bufs=1) as acts,
            tc.tile_pool(name="xt", bufs=4) as xt_pool,
            tc.tile_pool(name="qr", bufs=2) as qr_pool,
            tc.tile_pool(name="raw", bufs=3) as raw_pool,
            tc.tile_pool(name="tt", bufs=2) as t_pool,
            tc.tile_pool(name="est", bufs=4) as est_pool,
            tc.tile_pool(name="accp", bufs=4) as acc_pool,
            tc.tile_pool(name="rinv", bufs=2) as rinv_pool,
            tc.tile_pool(name="ysb", bufs=2) as y_pool,
            tc.tile_pool(name="osb", bufs=2) as o_pool,
            tc.tile_pool(name="pj", bufs=2, space="PSUM") as pj,
            tc.tile_pool(name="pst", bufs=2, space="PSUM") as pst,
            tc.tile_pool(name="py", bufs=2, space="PSUM") as py,
            tc.tile_pool(name="po", bufs=1, space="PSUM") as po,
        ):
            # ---- resident constants / weights (ACT HWDGE queue) ----
            cb_sb = consts.tile([P, 4, P], F16)
            wqkv_sb = consts.tile([P, 2, KT // 2, 4 * HD], F16)
            cs_sb = consts.tile([P, 2, T], F32)
            wo_sb = consts.tile([P, QH, D], F16)
            nc.scalar.dma_start(wqkv_sb[:, 0], wqkv_r[:, 0])
            nc.scalar.dma_start(wqkv_sb[:, 1], wqkv_r[:, 1])
            nc.scalar.dma_start(cs_sb, cf_r)
            nc.scalar.dma_start(wo_sb, wo_r)
            perm = cb_sb[:, 0, :]
            triu = cb_sb[:, 1, :]
            ident = cb_sb[:, 2, :]
            aones = cb_sb[:, 3, :]
            cos_t = cs_sb[:, 0, :]
            sin_t = cs_sb[:, 1, :]

            def wslc(kt, c0, c1):
                return wqkv_sb[:, kt // 8, kt % 8, c0:c1]

            # ---- resident activations (per-batch slots) ----
            kr_sb = acts.tile([P, B, T], F16)
            vt_sb = acts.tile([P, B, NJT_MAX, HD], F16)

            xt_tiles = {}

            def prefetch(hb):
                if hb >= NHB or hb in xt_tiles:
                    return
                xt = xt_pool.tile([P, KT // 2, NB], F16, tag="xt", name="xt")
                nc.sync.dma_start(xt, x_r[:, hb])
                xt_tiles[hb] = xt

            def rope(ps_raw, dst, t0):
                # dst(fp16) = raw*cos + swap(raw)*ssin; swap via PE perm matmul
                raw = raw_pool.tile([P, NB], F16, tag="raw")
                nc.scalar.copy(raw, ps_raw)  # frees the psum bank quickly
                t1 = t_pool.tile([P, NB], F32, tag="t1")
                nc.gpsimd.tensor_mul(t1, raw, cos_t[:, t0:t0 + NB])
                ps_sw = pst.tile([P, NB], F32, tag="st", name="ps_sw")
                nc.tensor.matmul(ps_sw, perm, raw, start=True, stop=True)
                t2 = t_pool.tile([P, NB], F32, tag="t2")
                nc.vector.tensor_mul(t2, ps_sw, sin_t[:, t0:t0 + NB])
                nc.vector.tensor_add(dst, t1, t2)

            def make_outproj_steps(i0p, y_prev, po_tiles):
                steps = []
                state = {}

                def step(s, jb):
                    def run(in_attn=True):
                        bank = po_tiles[(s * (D // NB) + jb) % len(po_tiles)]
                        if jb == 0:
                            state["o"] = o_pool.tile([P, D], F16, tag="o",
                                                     name="o_sb")
                        o_sb = state["o"]
                        nc.tensor.matmul(
                            bank,
                            y_prev[:, 0, s * P:(s + 1) * P],
                            wo_sb[:, 0, jb * NB:(jb + 1) * NB],
                            start=True, stop=False,
                        )
                        nc.tensor.matmul(
                            bank,
                            y_prev[:, 1, s * P:(s + 1) * P],
                            wo_sb[:, 1, jb * NB:(jb + 1) * NB],
                            start=False, stop=True,
                        )
                        dst = o_sb[:, jb * NB:(jb + 1) * NB]
                        # interleaved in attention ACT is exp-bound: give it
                        # only 1-in-4 copies; in bare tail flushes alternate
                        # DVE/ACT so the po ring drains at matmul rate
                        on_act = jb == 1 if in_attn else jb % 2 == 1
                        if on_act:
                            nc.scalar.copy(dst, bank)
                        else:
                            nc.vector.tensor_copy(dst, bank)
                        if jb == D // NB - 1:
                            row0 = i0p + s * P
                            nc.sync.dma_start(out_d[row0:row0 + P, :], o_sb)
                    return run

                for s in range(NB // P):
                    for jb in range(D // NB):
                        steps.append(step(s, jb))
                return steps

            def emit_proj(b, ib, gblk):
                xta = xt_tiles.pop(2 * gblk)
                xtb = xt_tiles.pop(2 * gblk + 1)
                prefetch(2 * gblk + 4)
                prefetch(2 * gblk + 5)
                t0 = ib * NB

                def xthalf(kt):
                    return (xta if kt < 8 else xtb)[:, kt % 8, :]

                # pass A: the two local q heads
                ps_q0 = pj.tile([P, NB], F32, tag="pj", name="ps_q0")
                ps_q1 = pj.tile([P, NB], F32, tag="pj", name="ps_q1")
                for kt in range(KT):
                    st, sp = kt == 0, kt == KT - 1
                    nc.tensor.matmul(ps_q0, wslc(kt, 0, P), xthalf(kt),
                                     start=st, stop=sp)
                    nc.tensor.matmul(ps_q1, wslc(kt, P, 2 * P), xthalf(kt),
                                     start=st, stop=sp)
                qr = qr_pool.tile([P, QH, NB], F16, tag="qr", name="qr")
                rope(ps_q0, qr[:, 0, :], t0)
                rope(ps_q1, qr[:, 1, :], t0)
                # pass B: k and v for the local kv head
                ps_k = pj.tile([P, NB], F32, tag="pj", name="ps_k")
                ps_v = pj.tile([P, NB], F32, tag="pj", name="ps_v")
                for kt in range(KT):
                    st, sp = kt == 0, kt == KT - 1
                    nc.tensor.matmul(ps_k, wslc(kt, 2 * P, 3 * P), xthalf(kt),
                                     start=st, stop=sp)
                    nc.tensor.matmul(ps_v, wslc(kt, 3 * P, 4 * P), xthalf(kt),
                                     start=st, stop=sp)
                rope(ps_k, kr_sb[:, b, ib * NB:(ib + 1) * NB], t0)
                vraw = raw_pool.tile([P, NB], F16, tag="raw", name="vraw")
                nc.scalar.copy(vraw, ps_v)
                ps_tr = pj.tile([P, 4, P], F16, tag="pj", name="ps_tr")
                for s4 in range(4):
                    nc.tensor.transpose(ps_tr[:, s4, :],
                                        vraw[:, s4 * P:(s4 + 1) * P], ident)
                nc.vector.tensor_copy(vt_sb[:, b, ib * 4:(ib + 1) * 4, :], ps_tr)
                return qr

            def emit_attn(b, ib, qr, steps):
                y_sb = y_pool.tile([P, QH, NB], F16, tag="y", name="y_sb")
                njt = 4 * ib + 4
                for h in range(QH):
                    ps_y = py.tile([P, NB], F32, tag="py", name="ps_y")
                    acc0 = acc_pool.tile([P, NB], F16, tag="acc", name="acc0")
                    acc1 = acc_pool.tile([P, NB], F16, tag="acc", name="acc1")

                    def consume(jt, a, sub, est):
                        # mask + denominator-chain add + PV for tile jt,
                        # emitted one tile late so PV's exp dependency has a
                        # full tile-time of slack (PE queue is in-order; an
                        # exp-stalled PV would block the next score matmul)
                        if a >= 0:  # diagonal tile: causal triangle mask
                            nc.vector.tensor_mul(est[:, sub:sub + P],
                                                 est[:, sub:sub + P], triu)
                        acc = acc0 if jt % 2 == 0 else acc1
                        if jt < 2:  # first tile of this chain
                            if sub > 0:
                                nc.vector.memset(acc[:, 0:sub], 0.0)
                            nc.vector.tensor_copy(acc[:, sub:], est[:, sub:])
                        else:
                            nc.vector.tensor_add(acc[:, sub:], acc[:, sub:],
                                                 est[:, sub:])
                        nc.tensor.matmul(
                            ps_y[:, sub:],
                            vt_sb[:, b, jt, :],
                            est[:, sub:],
                            start=jt == 0, stop=jt == njt - 1,
                        )
                        if steps:
                            steps.pop(0)()

                    pend = None
                    for jt in range(njt):
                        a = jt - 4 * ib
                        sub = max(0, a) * P
                        ps = pst.tile([P, NB], F32, tag="st", name="ps_st")
                        nc.tensor.matmul(
                            ps[:, sub:],
                            kr_sb[:, b, jt * P:(jt + 1) * P],
                            qr[:, h, sub:],
                            start=True, stop=True,
                        )
                        est = est_pool.tile([P, NB], F16, tag="est", name="est")
                        nc.scalar.activation(est[:, sub:], ps[:, sub:], EXP,
                                             scale=SCALE)
                        if pend is not None:
                            consume(*pend)
                        pend = (jt, a, sub, est)
                    consume(*pend)
                    # fused partition-reduce + broadcast: every row of the
                    # all-ones matmul output is the per-column denominator
                    rb_ps = pst.tile([P, NB], F32, tag="st", name="rb_ps")
                    nc.tensor.matmul(rb_ps, aones, acc0, start=True, stop=False)
                    nc.tensor.matmul(rb_ps, aones, acc1, start=False, stop=True)
                    rinv = rinv_pool.tile([P, NB], F32, tag="rinv", name="rinv")
                    nc.vector.reciprocal_approx_fast(rinv, rb_ps)
                    nc.vector.tensor_mul(y_sb[:, h, :], ps_y, rinv)
                return y_sb

            prefetch(0)
            prefetch(1)
            nc.sync.dma_start(cb_sb, cb_r)
            prefetch(2)
            prefetch(3)
            steps = []
            for b in range(B):
                for ib in range(IB):
                    gblk = b * IB + ib
                    qr = emit_proj(b, ib, gblk)
                    y_sb = emit_attn(b, ib, qr, steps)
                    for f in steps:  # leftovers (small-ib blocks)
                        f(in_attn=False)
                    po_t = po.tile([P, 2, NB], F32, tag="po", name="po_t")
                    po_tiles = [po_t[:, 0, :], po_t[:, 1, :]]
                    if (b, ib) == (B - 1, IB - 1):
                        # last block: projections are done, borrow the pj ring
                        # so the bare final out-proj rotates over 4 banks
                        po_tiles.append(pj.tile([P, NB], F32, tag="pj",
                                                name="po_t2"))
                        po_tiles.append(pj.tile([P, NB], F32, tag="pj",
                                                name="po_t3"))
                    steps = make_outproj_steps(b * T + ib * NB, y_sb, po_tiles)
            for f in steps:
                f(in_attn=False)

    nc.compile()
    return nc


def _host_prep(x, rope, wq, wk, wv, wo):
    """Build the 8 per-core input maps: shard, fp16, partition-major pack."""
    f16 = np.float16
    xT = x.reshape(BT, D).T.astype(f16)                 # [D, BT]
    xp = np.ascontiguousarray(
        xT.reshape(KT, P, 2 * IB, NB).transpose(1, 2, 0, 3).reshape(P, -1))
    cos = np.asarray(rope[..., 0], dtype=np.float32)    # [T, 64]
    sin = np.asarray(rope[..., 1], dtype=np.float32)
    cosT = np.concatenate([cos.T, cos.T], axis=0)       # [128, T]
    ssinT = np.concatenate([-sin.T, sin.T], axis=0)
    cf = np.ascontiguousarray(np.concatenate([cosT, ssinT], axis=1))
    permm = np.zeros((P, P), dtype=np.float32)
    permm[(np.arange(P) + 64) % P, np.arange(P)] = 1.0
    triu = np.triu(np.ones((P, P), dtype=np.float32))
    ident = np.eye(P, dtype=np.float32)
    aones = np.ones((P, P), dtype=np.float32)
    cb = np.ascontiguousarray(
        np.concatenate([permm, triu, ident, aones], axis=1).astype(f16))

    in_maps = []
    for c in range(NCORES):
        kv = c // 2
        wqkv = np.concatenate(
            [wq[QH * HD * c:QH * HD * (c + 1), :].T,
             wk[HD * kv:HD * (kv + 1), :].T,
             wv[HD * kv:HD * (kv + 1), :].T], axis=1).astype(f16)  # [D, 512]
        wqkv_p = np.ascontiguousarray(
            wqkv.reshape(KT, P, 4 * HD).transpose(1, 0, 2).reshape(P, -1))
        woT = wo[:, QH * HD * c:QH * HD * (c + 1)].T.astype(f16)   # [256, D]
        wo_p = np.ascontiguousarray(
            woT.reshape(QH, P, D).transpose(1, 0, 2).reshape(P, -1))
        in_maps.append(
            {"xp": xp, "wqkv": wqkv_p, "wop": wo_p, "cb": cb, "cf": cf}
        )
    return in_maps


LAST_RESULTS = None


def kernel(x, rope, wq, wk, wv, wo):
    global LAST_RESULTS
    from concourse import bass_utils

    if "nc" not in _CACHE:
        _CACHE["nc"] = _build()
    nc = _CACHE["nc"]

    in_maps = _host_prep(
        np.asarray(x), np.asarray(rope), np.asarray(wq), np.asarray(wk),
        np.asarray(wv), np.asarray(wo)
    )
    res = bass_utils.run_bass_kernel_spmd(nc, in_maps, core_ids=list(range(NCORES)))
    LAST_RESULTS = res
    acc = np.zeros((BT, D), dtype=np.float64)
    for c in range(NCORES):
        acc += res.results[c]["out"].astype(np.float64)
    return acc.reshape(B, T, D).astype(np.float32)


# revision 19
# speedup vs baseline: 1.0759x; 1.0039x over previous
"""GQA attention forward on 8 TRN2 NeuronCores, tensor-parallel across heads.

Problem (hardcoded): B=2, T=2048, D=2048, 16 q-heads, 4 kv-heads, head_dim=128,
RoPE (rotate-half pairing i <-> i+64), causal softmax, output projection.

Sharding (per core c of 8):
  q-heads 2c, 2c+1 (rows 256c:256c+256 of wq), kv-head c//2 (rows of wk/wv),
  wo input-dim slice [:, 256c:256c+256]. x replicated. Each core computes a
  full-shape partial of the output (y_local @ wo_slice.T); host sums partials.

v3 design notes:
  - fp16 activations/weights (f32 PSUM): all tensors here are O(100) so fp16's
    4x-finer mantissa beats bf16 at identical PE/DMA cost, and 16-bit DVE ops
    run in 2x mode. CPU-sim rel err 7e-4 (max exp value ~1.1e3 << 65504).
  - Host packs every DRAM tensor partition-major so each DMA is 128 contiguous
    descriptors (the naive [D, features] layouts produced 256B descriptors
    that made weight loads 5-9us each).
  - Fused pipeline per 512-token block: proj -> attention, with the previous
    block's out-projection emitted one jb-chunk at a time BETWEEN attention
    j-tiles (attention alone is exp-throughput-bound on ACT at ~690ns/tile vs
    the PE's ~430ns/tile, so out-proj matmuls fill the PE bubbles).
  - Softmax denominator: est tiles accumulate elementwise into two fp16
    chains (even/odd j-tiles, halving the serial DVE latency), then ONE
    all-ones matmul per chain fuses the partition-reduce AND the broadcast
    (every output row = column sums) into a 512-cycle PE op. reciprocal via
    the approx-fast custom DVE op. No gpsimd in the chain (its library swaps
    between op types cost ~15us stalls in v2); gpsimd only runs the rope cos
    muls (single op type, single library).
  - PSUM = exactly 8 banks: pj(2: q0/q1/k/v/vtr ring), pst(2: rope-swap +
    score tiles + denom ring), py(2), po(2, jb ping-pong in halves).
"""
import math
import numpy as np

P = 128
B = 2
T = 2048
D = 2048
BT = B * T            # 4096
HD = 128              # head dim
QH = 2                # local q heads per core
KT = D // P           # 16 contraction tiles over D
NB = 512              # free-dim block (tokens)
IB = T // NB          # 4 i-blocks per batch
NJT_MAX = T // P      # 16 j-tiles per batch
NCORES = 8
SCALE = 1.0 / math.sqrt(HD)

_CACHE = {}


def _build():
    import concourse.bass as bass
    import concourse.mybir as mybir
    from concourse import bacc
    from concourse.tile import TileContext

    F32 = mybir.dt.float32
    F16 = mybir.dt.float16
    EXP = mybir.ActivationFunctionType.Exp

    nc = bacc.Bacc("TRN2", target_bir_lowering=False, debug=False)

    # all inputs partition-major-packed on host: [128, ...] contiguous rows
    x_d = nc.dram_tensor("xp", [P, 2 * IB * KT * NB], F16, kind="ExternalInput").ap()
    wqkv_d = nc.dram_tensor("wqkv", [P, KT * 4 * HD], F16, kind="ExternalInput").ap()
    wo_d = nc.dram_tensor("wop", [P, QH * D], F16, kind="ExternalInput").ap()
    cb_d = nc.dram_tensor("cb", [P, 4 * P], F16, kind="ExternalInput").ap()
    cf_d = nc.dram_tensor("cf", [P, 2 * T], F32, kind="ExternalInput").ap()
    out_d = nc.dram_tensor("out", [BT, D], F16, kind="ExternalOutput").ap()

    NHB = 2 * IB * 2   # 16 half-blocks of 8 kt-tiles each
    x_r = x_d.rearrange("p (hb kt m) -> p hb kt m", hb=NHB, kt=KT // 2)
    wqkv_r = wqkv_d.rearrange("p (h kt m) -> p h kt m", h=2, kt=KT // 2)
    wo_r = wo_d.rearrange("p (h j) -> p h j", h=QH)
    cb_r = cb_d.rearrange("p (a q) -> p a q", a=4)
    cf_r = cf_d.rearrange("p (a t) -> p a t", a=2)

    with TileContext(nc) as tc:
        with (
            tc.tile_pool(name="consts", bufs=1) as consts,
            tc.tile_pool(name="acts", bufs=1) as acts,
            tc.tile_pool(name="xt", bufs=4) as xt_pool,
            tc.tile_pool(name="qr", bufs=2) as qr_pool,
            tc.tile_pool(name="raw", bufs=3) as raw_pool,
            tc.tile_pool(name="tt", bufs=2) as t_pool,
            tc.tile_pool(name="est", bufs=6) as est_pool,
            tc.tile_pool(name="accp", bufs=4) as acc_pool,
            tc.tile_pool(name="rinv", bufs=2) as rinv_pool,
            tc.tile_pool(name="ysb", bufs=2) as y_pool,
            tc.tile_pool(name="osb", bufs=2) as o_pool,
            tc.tile_pool(name="pj", bufs=2, space="PSUM") as pj,
            tc.tile_pool(name="pst", bufs=2, space="PSUM") as pst,
            tc.tile_pool(name="py", bufs=2, space="PSUM") as py,
            tc.tile_pool(name="po", bufs=1, space="PSUM") as po,
        ):
            # ---- resident constants / weights (ACT HWDGE queue) ----
            cb_sb = consts.tile([P, 4, P], F16)
            wqkv_sb = consts.tile([P, 2, KT // 2, 4 * HD], F16)
            cs_sb = consts.tile([P, 2, T], F32)
            wo_sb = consts.tile([P, QH, D], F16)
            nc.scalar.dma_start(wqkv_sb[:, 0], wqkv_r[:, 0])
            nc.scalar.dma_start(wqkv_sb[:, 1], wqkv_r[:, 1])
            nc.scalar.dma_start(cs_sb, cf_r)
            nc.scalar.dma_start(wo_sb, wo_r)
            perm = cb_sb[:, 0, :]
            triu = cb_sb[:, 1, :]
            ident = cb_sb[:, 2, :]
            aones = cb_sb[:, 3, :]
            cos_t = cs_sb[:, 0, :]
            sin_t = cs_sb[:, 1, :]

            def wslc(kt, c0, c1):
                return wqkv_sb[:, kt // 8, kt % 8, c0:c1]

            # ---- resident activations (per-batch slots) ----
            kr_sb = acts.tile([P, B, T], F16)
            vt_sb = acts.tile([P, B, NJT_MAX, HD], F16)

            xt_tiles = {}

            def prefetch(hb):
                if hb >= NHB or hb in xt_tiles:
                    return
                xt = xt_pool.tile([P, KT // 2, NB], F16, tag="xt", name="xt")
                nc.sync.dma_start(xt, x_r[:, hb])
                xt_tiles[hb] = xt

            def rope(ps_raw, dst, t0):
                # dst(fp16) = raw*cos + swap(raw)*ssin; swap via PE perm matmul
                raw = raw_pool.tile([P, NB], F16, tag="raw")
                nc.scalar.copy(raw, ps_raw)  # frees the psum bank quickly
                t1 = t_pool.tile([P, NB], F32, tag="t1")
                nc.gpsimd.tensor_mul(t1, raw, cos_t[:, t0:t0 + NB])
                ps_sw = pst.tile([P, NB], F32, tag="st", name="ps_sw")
                nc.tensor.matmul(ps_sw, perm, raw, start=True, stop=True)
                t2 = t_pool.tile([P, NB], F32, tag="t2")
                nc.vector.tensor_mul(t2, ps_sw, sin_t[:, t0:t0 + NB])
                nc.vector.tensor_add(dst, t1, t2)

            def make_outproj_steps(i0p, y_prev, po_tiles):
                steps = []
                state = {}

                def step(s, jb):
                    def run(in_attn=True):
                        bank = po_tiles[(s * (D // NB) + jb) % len(po_tiles)]
                        if jb == 0:
                            state["o"] = o_pool.tile([P, D], F16, tag="o",
                                                     name="o_sb")
                        o_sb = state["o"]
                        nc.tensor.matmul(
                            bank,
                            y_prev[:, 0, s * P:(s + 1) * P],
                            wo_sb[:, 0, jb * NB:(jb + 1) * NB],
                            start=True, stop=False,
                        )
                        nc.tensor.matmul(
                            bank,
                            y_prev[:, 1, s * P:(s + 1) * P],
                            wo_sb[:, 1, jb * NB:(jb + 1) * NB],
                            start=False, stop=True,
                        )
                        dst = o_sb[:, jb * NB:(jb + 1) * NB]
                        # interleaved in attention ACT is exp-bound: give it
                        # only 1-in-4 copies; in bare tail flushes alternate
                        # DVE/ACT so the po ring drains at matmul rate
                        on_act = jb == 1 if in_attn else jb % 2 == 1
                        if on_act:
                            nc.scalar.copy(dst, bank)
                        else:
                            nc.vector.tensor_copy(dst, bank)
                        if jb == D // NB - 1:
                            row0 = i0p + s * P
                            nc.sync.dma_start(out_d[row0:row0 + P, :], o_sb)
                    return run

                for s in range(NB // P):
                    for jb in range(D // NB):
                        steps.append(step(s, jb))
                return steps

            def emit_proj(b, ib, gblk):
                xta = xt_tiles.pop(2 * gblk)
                xtb = xt_tiles.pop(2 * gblk + 1)
                prefetch(2 * gblk + 4)
                prefetch(2 * gblk + 5)
                t0 = ib * NB

                def xthalf(kt):
                    return (xta if kt < 8 else xtb)[:, kt % 8, :]

                # pass A: the two local q heads
                ps_q0 = pj.tile([P, NB], F32, tag="pj", name="ps_q0")
                ps_q1 = pj.tile([P, NB], F32, tag="pj", name="ps_q1")
                for kt in range(KT):
                    st, sp = kt == 0, kt == KT - 1
                    nc.tensor.matmul(ps_q0, wslc(kt, 0, P), xthalf(kt),
                                     start=st, stop=sp)
                    nc.tensor.matmul(ps_q1, wslc(kt, P, 2 * P), xthalf(kt),
                                     start=st, stop=sp)
                qr = qr_pool.tile([P, QH, NB], F16, tag="qr", name="qr")
                rope(ps_q0, qr[:, 0, :], t0)
                rope(ps_q1, qr[:, 1, :], t0)
                # pass B: k and v for the local kv head
                ps_k = pj.tile([P, NB], F32, tag="pj", name="ps_k")
                ps_v = pj.tile([P, NB], F32, tag="pj", name="ps_v")
                for kt in range(KT):
                    st, sp = kt == 0, kt == KT - 1
                    nc.tensor.matmul(ps_k, wslc(kt, 2 * P, 3 * P), xthalf(kt),
                                     start=st, stop=sp)
                    nc.tensor.matmul(ps_v, wslc(kt, 3 * P, 4 * P), xthalf(kt),
                                     start=st, stop=sp)
                rope(ps_k, kr_sb[:, b, ib * NB:(ib + 1) * NB], t0)
                vraw = raw_pool.tile([P, NB], F16, tag="raw", name="vraw")
                nc.scalar.copy(vraw, ps_v)
                ps_tr = pj.tile([P, 4, P], F16, tag="pj", name="ps_tr")
                for s4 in range(4):
                    nc.tensor.transpose(ps_tr[:, s4, :],
                                        vraw[:, s4 * P:(s4 + 1) * P], ident)
                nc.vector.tensor_copy(vt_sb[:, b, ib * 4:(ib + 1) * 4, :], ps_tr)
                return qr

            def emit_attn(b, ib, qr, steps):
                y_sb = y_pool.tile([P, QH, NB], F16, tag="y", name="y_sb")
                njt = 4 * ib + 4
                for h in range(QH):
                    ps_y = py.tile([P, NB], F32, tag="py", name="ps_y")
                    acc0 = acc_pool.tile([P, NB], F16, tag="acc", name="acc0")
                    acc1 = acc_pool.tile([P, NB], F16, tag="acc", name="acc1")

                    def consume(jt, a, sub, est):
                        # mask + denominator-chain add + PV for tile jt,
                        # emitted one tile late so PV's exp dependency has a
                        # full tile-time of slack (PE queue is in-order; an
                        # exp-stalled PV would block the next score matmul)
                        if a >= 0:  # diagonal tile: causal triangle mask
                            nc.vector.tensor_mul(est[:, sub:sub + P],
                                                 est[:, sub:sub + P], triu)
                        acc = acc0 if jt % 2 == 0 else acc1
                        if jt < 2:  # first tile of this chain
                            if sub > 0:
                                nc.vector.memset(acc[:, 0:sub], 0.0)
                            nc.vector.tensor_copy(acc[:, sub:], est[:, sub:])
                        else:
                            nc.vector.tensor_add(acc[:, sub:], acc[:, sub:],
                                                 est[:, sub:])
                        nc.tensor.matmul(
                            ps_y[:, sub:],
                            vt_sb[:, b, jt, :],
                            est[:, sub:],
                            start=jt == 0, stop=jt == njt - 1,
                        )
                        if steps:
                            steps.pop(0)()

                    pend = None
                    for jt in range(njt):
                        a = jt - 4 * ib
                        sub = max(0, a) * P
                        ps = pst.tile([P, NB], F32, tag="st", name="ps_st")
                        nc.tensor.matmul(
                            ps[:, sub:],
                            kr_sb[:, b, jt * P:(jt + 1) * P],
                            qr[:, h, sub:],
                            start=True, stop=True,
                        )
                        est = est_pool.tile([P, NB], F16, tag="est", name="est")
                        nc.scalar.activation(est[:, sub:], ps[:, sub:], EXP,
                                             scale=SCALE)
                        if pend is not None:
                            consume(*pend)
                        pend = (jt, a, sub, est)
                    consume(*pend)
                    # fused partition-reduce + broadcast: every row of the
                    # all-ones matmul output is the per-column denominator
                    rb_ps = pst.tile([P, NB], F32, tag="st", name="rb_ps")
                    nc.tensor.matmul(rb_ps, aones, acc0, start=True, stop=False)
                    nc.tensor.matmul(rb_ps, aones, acc1, start=False, stop=True)
                    rinv = rinv_pool.tile([P, NB], F32, tag="rinv", name="rinv")
                    nc.vector.reciprocal_approx_fast(rinv, rb_ps)
                    nc.vector.tensor_mul(y_sb[:, h, :], ps_y, rinv)
                return y_sb

            prefetch(0)
            prefetch(1)
            nc.sync.dma_start(cb_sb, cb_r)
            prefetch(2)
            prefetch(3)
            steps = []
            for b in range(B):
                for ib in range(IB):
                    gblk = b * IB + ib
                    qr = emit_proj(b, ib, gblk)
                    y_sb = emit_attn(b, ib, qr, steps)
                    for f in steps:  # leftovers (small-ib blocks)
                        f(in_attn=False)
                    po_t = po.tile([P, 2, NB], F32, tag="po", name="po_t")
                    po_tiles = [po_t[:, 0, :], po_t[:, 1, :]]
                    if (b, ib) == (B - 1, IB - 1):
                        # last block: projections are done, borrow the pj ring
                        # so the bare final out-proj rotates over 4 banks
                        po_tiles.append(pj.tile([P, NB], F32, tag="pj",
                                                name="po_t2"))
                        po_tiles.append(pj.tile([P, NB], F32, tag="pj",
                                                name="po_t3"))
                    steps = make_outproj_steps(b * T + ib * NB, y_sb, po_tiles)
            for f in steps:
                f(in_attn=False)

    nc.compile()
    return nc


def _host_prep(x, rope, wq, wk, wv, wo):
    """Build the 8 per-core input maps: shard, fp16, partition-major pack."""
    f16 = np.float16
    xT = x.reshape(BT, D).T.astype(f16)                 # [D, BT]
    xp = np.ascontiguousarray(
        xT.reshape(KT, P, 2 * IB, NB).transpose(1, 2, 0, 3).reshape(P, -1))
    cos = np.asarray(rope[..., 0], dtype=np.float32)    # [T, 64]
    sin = np.asarray(rope[..., 1], dtype=np.float32)
    cosT = np.concatenate([cos.T, cos.T], axis=0)       # [128, T]
    ssinT = np.concatenate([-sin.T, sin.T], axis=0)
    cf = np.ascontiguousarray(np.concatenate([cosT, ssinT], axis=1))
    permm = np.zeros((P, P), dtype=np.float32)
    permm[(np.arange(P) + 64) % P, np.arange(P)] = 1.0
    triu = np.triu(np.ones((P, P), dtype=np.float32))
    ident = np.eye(P, dtype=np.float32)
    aones = np.ones((P, P), dtype=np.float32)
    cb = np.ascontiguousarray(
        np.concatenate([permm, triu, ident, aones], axis=1).astype(f16))

    in_maps = []
    for c in range(NCORES):
        kv = c // 2
        wqkv = np.concatenate(
            [wq[QH * HD * c:QH * HD * (c + 1), :].T,
             wk[HD * kv:HD * (kv + 1), :].T,
             wv[HD * kv:HD * (kv + 1), :].T], axis=1).astype(f16)  # [D, 512]
        wqkv_p = np.ascontiguousarray(
            wqkv.reshape(KT, P, 4 * HD).transpose(1, 0, 2).reshape(P, -1))
        woT = wo[:, QH * HD * c:QH * HD * (c + 1)].T.astype(f16)   # [256, D]
        wo_p = np.ascontiguousarray(
            woT.reshape(QH, P, D).transpose(1, 0, 2).reshape(P, -1))
        in_maps.append(
            {"xp": xp, "wqkv": wqkv_p, "wop": wo_p, "cb": cb, "cf": cf}
        )
    return in_maps


LAST_RESULTS = None


def kernel(x, rope, wq, wk, wv, wo):
    global LAST_RESULTS
    from concourse import bass_utils

    if "nc" not in _CACHE:
        _CACHE["nc"] = _build()
    nc = _CACHE["nc"]

    in_maps = _host_prep(
        np.asarray(x), np.asarray(rope), np.asarray(wq), np.asarray(wk),
        np.asarray(wv), np.asarray(wo)
    )
    res = bass_utils.run_bass_kernel_spmd(nc, in_maps, core_ids=list(range(NCORES)))
    LAST_RESULTS = res
    acc = np.zeros((BT, D), dtype=np.float64)
    for c in range(NCORES):
        acc += res.results[c]["out"].astype(np.float64)
    return acc.reshape(B, T, D).astype(np.float32)
